# revision 22
# baseline (speedup 1.0000x reference)
"""BiLSTM-CRF NLL kernel for 8 Trainium2 NeuronCores.

Contract: kernel(**inputs) takes the FULL unsharded inputs (as produced by the
reference setup_inputs()) and returns the FULL output (a float32 scalar).

Sharding (hardcoded): data-parallel over batch. B=64 -> 8 shards of 8
sequences; params replicated. Each core computes sum_b(logZ_b) - sum emis[gold]
for its 8 sequences; the host adds the (host-computable) gold
transition/start/end score and sums the 8 partial scalars.

Per-core pipeline:
  0. embedding gather via indirect DMA ([128, E] token chunks), PE-transposed
     to xT [E, tokens] in bf16.
  1. input projections g_ih = W_ih @ x + b, bf16 matmuls with N=256,
     stored f16, gate chunks permuted to (i,i,f,f,o,o,g,g), g rows x2 extra.
     Emission is PACED: ~2 PE ops per recurrence step so phase-1 never stalls
     the recurrence chain.
  2. the two LSTM recurrences interleaved. Per step and direction: one f16
     identity-matmul preloads g_ih into PSUM (one step ahead); 16 bf16
     [128,128]x[128,8] matmuls accumulate W_hh @ h (these pipeline at ~27ns
     pitch on the PE); sigmoid on ACT covers all four gates (tanh(g) via
     2*sig(2g)-1); cell update on DVE; tanh on ACT; h = o*tanh(c) written
     bf16 by GPSIMD straight into the k-major h history buffer that serves
     as both the next-step matmul operand and the emission operand.
  3. emissions transposed [9, tokens]: per 512-token group four bf16 matmuls
     (dir x k-plane, contiguous rhs thanks to the k-major h layout);
     E = exp(psum + b_tag - mu) in one ACT; raw emissions = psum + b_tag
     on DVE (for the gold score).
  4. gold emission score: (emisraw * onehot) with accum_out on GPSIMD,
     overlapped with the CRF chains.
  5. CRF in exp space, bf16, as FOUR concurrent chains (serial depth 84):
     alpha (t=0..83) and beta (t=255..172) vector chains plus two per-batch
     9x9 transfer-matrix product chains covering the middle; the combine
     contracts alpha^T G1^T G2^T (M beta) with two lhsT-matmuls using a
     diagonal-batch mask + selector matmul. logZ = log(sum) + 256*mu.
     The gold emission score runs on GPSIMD concurrently with the CRF.
"""

import functools
import math
import os
import sys

import numpy as np

for _p in ("/opt/trn_rl_repo", "/opt/pypackages"):
    if _p not in sys.path and os.path.isdir(_p):
        sys.path.append(_p)

import ml_dtypes  # noqa: E402

import concourse.bass as bass  # noqa: E402
import concourse.mybir as mybir  # noqa: E402
import concourse.tile as tile  # noqa: E402
from concourse import bacc  # noqa: E402
from concourse.bass import IndirectOffsetOnAxis  # noqa: E402
from concourse.bass_utils import run_bass_kernel_spmd  # noqa: E402

F32 = mybir.dt.float32
F16 = mybir.dt.float16
BF16 = mybir.dt.bfloat16
FP8 = mybir.dt.float8e4
I32 = mybir.dt.int32
AF = mybir.ActivationFunctionType
OP = mybir.AluOpType
DR = mybir.MatmulPerfMode.DoubleRowSwInterleave
NPFP8 = mybir.dt.np(FP8)
NPBF16 = ml_dtypes.bfloat16

# Problem constants (hardcoded per the task contract).
B, S, V, E, H, T = 64, 256, 50000, 256, 512, 9
HD = H // 2               # 256 per-direction hidden
NCORES = 8
BL = B // NCORES          # 8 sequences per core
TOK = BL * S              # 2048 tokens per core
NCH = TOK // 128          # 16 gather chunks of 128 tokens
MU = math.log(9.0)        # exp-space drift compensation, cancels exactly
# gate chunk permutation: original (i0 i1 f0 f1 g0 g1 o0 o1) -> (i i f f o o g g)
PERM = [0, 1, 2, 3, 6, 7, 4, 5]
SLOTS = S + 1             # h history slots per direction


def _hslot(hall_v, s):
    """[128, 2, 8] view of history slot s (k-major hall)."""
    return hall_v[:, :, s, :]


def _emit_preload(nc, d, t, gih, idf16, ps_pool):
    """Start step-t PSUM with g_ih(+bias) via identity matmul (h-independent)."""
    ps = ps_pool[d].tile([128, 64], F32, tag=f"st{d}", name=f"ps{d}")
    nc.tensor.matmul(
        out=ps[:, :],
        lhsT=idf16[:],
        rhs=gih[d][:, t * 64:(t + 1) * 64],
        start=True,
        stop=False,
        skip_group_check=True,
    )
    return ps


def _emit_wmms(nc, d, t, ps, whhv, hall_v):
    rd = t if d == "f" else t + 1
    rhs = _hslot(hall_v[d], rd)
    for m in range(8):
        for k in range(2):
            nc.tensor.matmul(
                out=ps[:, m * 8:(m + 1) * 8],
                lhsT=whhv[d][k][:, m * 128:(m + 1) * 128],
                rhs=rhs[:, k],
                start=False,
                stop=(m == 7 and k == 1),
                skip_group_check=True,
            )


def _emit_tail(nc, d, t, ps, hall_v, c_state, work):
    wr = t + 1 if d == "f" else t
    # g-gate preacts are pre-scaled x2 on host: one sigmoid covers all four
    # gates: sig(i), sig(f), sig(o), sig(2g); tanh(g) = 2*sig(2g) - 1.
    sig = work.tile([128, 64], BF16, tag=f"sig{d}", name=f"sig{d}")
    nc.scalar.activation(sig[:], ps[:, :], AF.Sigmoid)
    u = work.tile([128, 16], BF16, tag=f"u{d}", name=f"u{d}")
    nc.vector.scalar_tensor_tensor(
        u[:], sig[:, 48:64], 0.5, sig[:, 0:16], op0=OP.subtract, op1=OP.mult
    )
    v = work.tile([128, 16], BF16, tag=f"v{d}", name=f"v{d}")
    nc.vector.tensor_tensor(v[:], sig[:, 16:32], c_state[d][:], op=OP.mult)
    nc.vector.scalar_tensor_tensor(
        c_state[d][:], u[:], 2.0, v[:], op0=OP.mult, op1=OP.add
    )
    tcn = work.tile([128, 16], BF16, tag=f"tc{d}", name=f"tc{d}")
    nc.scalar.activation(tcn[:], c_state[d][:], AF.Tanh)
    ogate = sig[:].rearrange("p (m k b) -> p m k b", m=4, k=2, b=8)[:, 2]
    tcv = tcn[:].rearrange("p (k b) -> p k b", k=2, b=8)
    nc.vector.tensor_tensor(_hslot(hall_v[d], wr), ogate, tcv, op=OP.mult)
    return tcn


@functools.lru_cache(maxsize=2)
def _build(seq_len=S):
    """Build the Bass program (same SPMD program for all 8 cores)."""
    assert seq_len == S, "builder is specialized to S=256"

    nc = bacc.Bacc("TRN2", target_bir_lowering=False, debug=False)

    # ---- DRAM I/O ----
    emb_d = nc.dram_tensor("emb", [V, E], F32, kind="ExternalInput")
    idx_d = nc.dram_tensor("idx", [128, NCH], I32, kind="ExternalInput")
    wih_d = {d: nc.dram_tensor(f"wih_{d}", [E, 4 * HD], BF16, kind="ExternalInput")
             for d in "fb"}
    whh_d = {d: nc.dram_tensor(f"whh_{d}", [HD, 4 * HD], BF16, kind="ExternalInput")
             for d in "fb"}
    br_d = {d: nc.dram_tensor(f"br_{d}", [128, 8], F32, kind="ExternalInput")
            for d in "fb"}
    wtag_d = nc.dram_tensor("wtag8", [128, 2 * 2 * T], BF16, kind="ExternalInput")
    btag_d = nc.dram_tensor("btag", [T, 1], F32, kind="ExternalInput")
    btagmu_d = nc.dram_tensor("btagmu", [T, 1], F32, kind="ExternalInput")
    expT_d = nc.dram_tensor("expT", [T, T], BF16, kind="ExternalInput")
    expTT_d = nc.dram_tensor("expTT", [T, T], BF16, kind="ExternalInput")
    exps_d = nc.dram_tensor("exps", [T, 1], F32, kind="ExternalInput")
    expe_d = nc.dram_tensor("expe", [T, 1], F32, kind="ExternalInput")
    ohc_d = nc.dram_tensor("ohc", [T, TOK], F32, kind="ExternalInput")
    selm_d = nc.dram_tensor("selm", [72, T], BF16, kind="ExternalInput")
    dgm_d = nc.dram_tensor("dgm", [72, 8], BF16, kind="ExternalInput")
    idf32_d = nc.dram_tensor("idf32", [128, 128], F32, kind="ExternalInput")
    idf16_d = nc.dram_tensor("idf16", [128, 128], F16, kind="ExternalInput")
    out_d = nc.dram_tensor("out", [1, 1], F32, kind="ExternalOutput")

    with tile.TileContext(nc) as tc:
        with (
            tc.tile_pool(name="pers", bufs=1) as pers,
            tc.tile_pool(name="work", bufs=3) as work,
            tc.tile_pool(name="psbig", bufs=2, space="PSUM") as ps_big,
            tc.tile_pool(name="pstp", bufs=2, space="PSUM") as ps_tp,
            tc.tile_pool(name="psf", bufs=2, space="PSUM") as ps_f,
            tc.tile_pool(name="psb", bufs=2, space="PSUM") as ps_b,
        ):
            ps_pool = {"f": ps_f, "b": ps_b}

            # ---- persistent SBUF ----
            idx_sb = pers.tile([128, NCH], I32, tag="idx")
            nc.sync.dma_start(idx_sb[:], idx_d[:])
            # gathers first: the GPSIMD queue starts with the indirect DMAs
            xg = pers.tile([128, NCH * E], F32, tag="xg")
            xT = [pers.tile([128, TOK], BF16, tag=f"xT{k}", name=f"xT{k}")
                  for k in range(2)]
            gorder = []
            for j in range(NCH // 4):
                gorder += [2 * j, 2 * j + 1, NCH - 2 - 2 * j, NCH - 1 - 2 * j]
            for ch in gorder:
                nc.gpsimd.indirect_dma_start(
                    out=xg[:, ch * E:(ch + 1) * E],
                    out_offset=None,
                    in_=emb_d[:],
                    in_offset=IndirectOffsetOnAxis(ap=idx_sb[:, ch:ch + 1], axis=0),
                )
            idf32 = pers.tile([128, 128], F32, tag="idf32")
            nc.sync.dma_start(idf32[:], idf32_d[:])
            idf16 = pers.tile([128, 128], F16, tag="idf16")
            nc.sync.dma_start(idf16[:], idf16_d[:])

            wih, whh, whhv, br, gih, hall, hall_v, c_state = ({} for _ in range(8))
            for d in "fb":
                wih[d] = [pers.tile([128, 4 * HD], BF16, tag=f"wih{d}{k}",
                                    name=f"wih{d}{k}") for k in range(2)]
                for k in range(2):
                    nc.sync.dma_start(wih[d][k][:], wih_d[d][k * 128:(k + 1) * 128, :])
                whh[d] = [pers.tile([128, 4 * HD], BF16, tag=f"whh{d}{k}",
                                    name=f"whh{d}{k}") for k in range(2)]
                for k in range(2):
                    nc.sync.dma_start(whh[d][k][:], whh_d[d][k * 128:(k + 1) * 128, :])
                whhv[d] = whh[d]
                br[d] = pers.tile([128, 8], F32, tag=f"br{d}", name=f"br{d}")
                nc.sync.dma_start(br[d][:], br_d[d][:])
                gih[d] = pers.tile([128, S * 64], F16, tag=f"gih{d}", name=f"gih{d}")
                hall[d] = pers.tile([128, 2 * SLOTS * 8], BF16, tag=f"hall{d}",
                                    name=f"hall{d}")
                hall_v[d] = hall[d][:].rearrange("p (k s b) -> p k s b",
                                                 k=2, s=SLOTS, b=8)
                c_state[d] = pers.tile([128, 16], F32, tag=f"c{d}", name=f"c{d}")
                nc.vector.memset(c_state[d][:], 0.0)
            # zero initial h slots (fwd reads slot 0, bwd reads slot S)
            nc.vector.memset(_hslot(hall_v["f"], 0), 0.0)
            nc.vector.memset(_hslot(hall_v["b"], S), 0.0)

            wtag8 = pers.tile([128, 2 * 2 * T], BF16, tag="wtag8")
            nc.sync.dma_start(wtag8[:], wtag_d[:])
            wtagv = wtag8[:].rearrange("p (d k t) -> p d k t", d=2, k=2, t=T)
            btag = pers.tile([T, 1], F32, tag="btag")
            nc.sync.dma_start(btag[:], btag_d[:])
            btagmu = pers.tile([T, 1], F32, tag="btagmu")
            nc.sync.dma_start(btagmu[:], btagmu_d[:])
            expTm = pers.tile([T, T], BF16, tag="expTm")
            nc.sync.dma_start(expTm[:], expT_d[:])
            expTTm = pers.tile([T, T], BF16, tag="expTTm")
            nc.sync.dma_start(expTTm[:], expTT_d[:])
            exps = pers.tile([T, 1], F32, tag="exps")
            nc.sync.dma_start(exps[:], exps_d[:])
            expe = pers.tile([T, 1], F32, tag="expe")
            nc.sync.dma_start(expe[:], expe_d[:])
            ohc = pers.tile([T, TOK], F32, tag="ohc")
            nc.sync.dma_start(ohc[:], ohc_d[:])
            ones9 = pers.tile([T, 1], F32, tag="ones9")
            nc.vector.memset(ones9[:], 1.0)
            # warm the Exp/Ln ACT tables so the scheduler's mid-loop hoist of
            # the emission exp never pays a table load on the critical path
            warm = pers.tile([1, 1], F32, tag="warm")
            nc.scalar.activation(warm[:], ones9[0:1, 0:1], AF.Exp)
            nc.scalar.activation(warm[:], ones9[0:1, 0:1], AF.Ln)
            selm = pers.tile([72, T], BF16, tag="selm")
            nc.sync.dma_start(selm[:], selm_d[:])
            dgm = pers.tile([72, 8], BF16, tag="dgm")
            nc.sync.dma_start(dgm[:], dgm_d[:])

            # ---- phase 1 as a paced op queue ----
            # chunk = 256 tokens = 32 t-slots; ops are closures, ~1 PE op each.
            transposed = set()

            def ops_transpose(gch):
                if gch in transposed:
                    return
                transposed.add(gch)
                for k in range(2):
                    def op(gch=gch, k=k):
                        pst = ps_tp.tile([128, 128], F32, tag="tp", name="tp")
                        nc.tensor.transpose(
                            out=pst[:],
                            in_=xg[:, gch * E + k * 128: gch * E + (k + 1) * 128],
                            identity=idf32[:],
                        )
                        nc.vector.tensor_copy(xT[k][:, gch * 128:(gch + 1) * 128],
                                              pst[:])
                    yield op

            def ops_chunk(d, c, half=None):
                # 256-token chunk c (t-slots 32c..32c+31), direction d.
                # half=0/1 emits only the first/second 128 tokens (16 slots).
                gchs = [2 * c, 2 * c + 1] if half is None else [2 * c + half]
                for g in gchs:
                    yield from ops_transpose(g)
                ntok = 256 if half is None else 128
                t00 = c * 32 + (0 if not half else 16)
                c00 = c * 256 + (0 if not half else 128)
                for m in range(8):
                    def op(d=d, m=m, ntok=ntok, t00=t00, c00=c00):
                        psg = ps_big.tile([128, ntok], F32, tag="big", name="psg")
                        for k in range(2):
                            nc.tensor.matmul(
                                out=psg[:],
                                lhsT=wih[d][k][:, m * 128:(m + 1) * 128],
                                rhs=xT[k][:, c00:c00 + ntok],
                                start=(k == 0),
                                stop=(k == 1),
                            )
                        dst = gih[d][:].rearrange(
                            "p (t m b) -> p t m b", t=S, m=8, b=8
                        )[:, t00:t00 + ntok // 8, m, :]
                        srcv = psg[:].rearrange("p (t b) -> p t b",
                                                t=ntok // 8, b=8)
                        if m % 2 == 0:
                            nc.vector.tensor_scalar_add(dst, srcv, br[d][:, m:m + 1])
                        else:
                            nc.scalar.activation(dst, srcv, AF.Identity,
                                                 bias=br[d][:, m:m + 1])
                    yield op

            NCHK = S // 32  # 8 chunks per direction
            # prelude: first fwd chunk + last bwd chunk fully
            # (list before building the queue: listing marks transposes done)
            prelude = (list(ops_chunk("f", 0)) + list(ops_chunk("b", NCHK - 1))
                       + list(ops_chunk("f", 1)) + list(ops_chunk("b", NCHK - 2)))
            p1_queue = []
            for j in range(2, NCHK):
                p1_queue += list(ops_chunk("f", j))
                p1_queue += list(ops_chunk("b", NCHK - 1 - j))
            p1_pos = 0
            for op in prelude:
                op()

            # ---- phase 2: the two LSTM recurrences, phase-1 paced in ----
            ps_cur = {"f": _emit_preload(nc, "f", 0, gih, idf16, ps_pool),
                      "b": _emit_preload(nc, "b", S - 1, gih, idf16, ps_pool)}
            prev_tcn = {}
            for t in range(S):
                for d in "fb":
                    tok = t if d == "f" else S - 1 - t
                    if d in prev_tcn:
                        # warm-keeper: depends on the previous step's tanh, so
                        # it executes just before h lands, keeping PE p-state
                        # hot for the W block (first MM 165ns cold vs ~27 warm)
                        psW = ps_tp.tile([128, 16], F32, tag="tp", name="psW")
                        nc.tensor.matmul(out=psW[:], lhsT=idf16[:],
                                         rhs=prev_tcn[d][:], start=True,
                                         stop=True, skip_group_check=True)
                    _emit_wmms(nc, d, tok, ps_cur[d], whhv, hall_v)
                for _ in range(2):
                    if p1_pos < len(p1_queue):
                        p1_queue[p1_pos]()
                        p1_pos += 1
                ps_nxt = {}
                if t + 1 < S:
                    ps_nxt = {
                        "f": _emit_preload(nc, "f", t + 1, gih, idf16, ps_pool),
                        "b": _emit_preload(nc, "b", S - 2 - t, gih, idf16,
                                           ps_pool),
                    }
                for d in "fb":
                    tok = t if d == "f" else S - 1 - t
                    prev_tcn[d] = _emit_tail(nc, d, tok, ps_cur[d], hall_v,
                                             c_state, work)
                ps_cur = ps_nxt

            # ---- phase 3: emissions via fp8 DoubleRow + fused exp ----
            emisraw = pers.tile([T, TOK], F32, tag="emisraw")
            ebuf = pers.tile([T, TOK], F32, tag="ebuf")
            hflat = {d: hall[d][:].rearrange("p (k sb) -> p k sb", k=2)
                     for d in "fb"}
            for n in range(4):
                pse = ps_big.tile([T, 512], F32, tag="big", name="pse")
                # fwd h_t lives at slot t+1; bwd h_t at slot t
                lo = {"f": (n * 64 + 1) * 8, "b": (n * 64) * 8}
                for di, d in enumerate("fb"):
                    for k in range(2):
                        nc.tensor.matmul(
                            out=pse[:], lhsT=wtagv[:, di, k],
                            rhs=hflat[d][:, k, lo[d]:lo[d] + 512],
                            start=(di == 0 and k == 0),
                            stop=(di == 1 and k == 1),
                        )
                nc.scalar.activation(ebuf[:, n * 512:(n + 1) * 512], pse[:],
                                     AF.Exp, bias=btagmu[:, 0:1])
                nc.vector.tensor_scalar_add(
                    emisraw[:, n * 512:(n + 1) * 512], pse[:], btag[:, 0:1]
                )

            # ---- phase 4: gold emission score on GPSIMD (overlaps CRF) ----
            gjunk = pers.tile([T, TOK], F32, tag="gjunk")
            for n in range(4):
                nc.gpsimd.tensor_tensor(
                    gjunk[:, n * 512:(n + 1) * 512],
                    emisraw[:, n * 512:(n + 1) * 512],
                    ohc[:, n * 512:(n + 1) * 512], op=OP.mult,
                )

            # ---- phase 5: CRF, four concurrent chains (depth 64) ----
            # A: alpha over t=0..63.  B: beta over t=255..192.
            # M1/M2: per-batch 9x9 transfer products over t=64..127 / 128..191,
            # stored [9, (j0, b)] j-major.  alpha_191 = P2 P1 a63;
            # logZ = sum_i alpha_191[i] * (M b_192)[i].
            e3 = ebuf[:].rearrange("p (t b) -> p t b", t=S, b=8)
            NQA = 84   # A/B chain depth (cheap rounds)
            NQM1 = 44  # M1 depth
            NQM2 = S - 2 * NQA - NQM1  # 44: M2 depth
            tmpAB = work.tile([T, 16], BF16, tag="tmpAB")
            nc.vector.tensor_scalar(
                tmpAB[:, 0:8], ebuf[:, 0:8], scalar1=exps[:, 0:1], scalar2=None,
                op0=OP.mult,
            )
            nc.vector.tensor_scalar(
                tmpAB[:, 8:16], ebuf[:, (S - 1) * 8:S * 8],
                scalar1=expe[:, 0:1], scalar2=None, op0=OP.mult,
            )
            # seeds: P = M^T * E_t (broadcast over j0 / b), t = 64 and 128
            pboth = work.tile([T, 144], BF16, tag="pboth")
            mtb = expTTm[:].unsqueeze(2).broadcast_to([T, T, 8])
            for h_, t0 in ((0, NQA), (72, NQA + NQM1)):
                eb = e3[:, t0:t0 + 1, :].broadcast_to([T, T, 8])
                nc.vector.tensor_tensor(
                    pboth[:, h_:h_ + 72].rearrange("p (j b) -> p j b", j=T, b=8),
                    mtb, eb, op=OP.mult,
                )
            for i in range(1, NQA):  # A: t=i; B: t=255-i; M: shorter chains
                tA = i
                tB = S - 1 - i
                psAB = ps_f.tile([T, 16], F32, tag="stf", name="psAB")
                nc.tensor.matmul(out=psAB[:, 0:8], lhsT=expTm[:],
                                 rhs=tmpAB[:, 0:8], start=True, stop=True)
                nc.tensor.matmul(out=psAB[:, 8:16], lhsT=expTTm[:],
                                 rhs=tmpAB[:, 8:16], start=True, stop=True)
                do1, do2 = i < NQM1, i < NQM2
                if do1 or do2:
                    psM = ps_big.tile([T, 144], F32, tag="big", name="psM")
                    if do1:
                        nc.tensor.matmul(out=psM[:, 0:72], lhsT=expTm[:],
                                         rhs=pboth[:, 0:72], start=True, stop=True)
                    if do2:
                        nc.tensor.matmul(out=psM[:, 72:144], lhsT=expTm[:],
                                         rhs=pboth[:, 72:144], start=True, stop=True)
                tmpAB = work.tile([T, 16], BF16, tag="tmpAB")
                nc.vector.tensor_tensor(
                    tmpAB[:], psAB[:], e3[:, tA:tB + 1:(tB - tA), :],
                    op=OP.mult,
                )
                if do1 or do2:
                    pprev, pboth = pboth, work.tile([T, 144], BF16, tag="pboth")
                    for q, do in ((0, do1), (1, do2)):
                        if not do:
                            nc.vector.tensor_copy(
                                pboth[:, q * 72:(q + 1) * 72],
                                pprev[:, q * 72:(q + 1) * 72])
                            continue
                        t_ = NQA + q * NQM1 + i
                        eM = e3[:, t_:t_ + 1, :]
                        eMb = eM.broadcast_to([T, T, 8])
                        nc.vector.tensor_tensor(
                            pboth[:, q * 72:(q + 1) * 72].rearrange(
                                "p (j b) -> p j b", j=T, b=8),
                            psM[:, q * 72:(q + 1) * 72].rearrange(
                                "p (j b) -> p j b", j=T, b=8),
                            eMb, op=OP.mult,
                        )
            # bp = M @ b_192
            psB = ps_b.tile([T, 8], F32, tag="stb")
            nc.tensor.matmul(out=psB[:], lhsT=expTTm[:], rhs=tmpAB[:, 8:16],
                             start=True, stop=True)
            bps = work.tile([T, 8], BF16, tag="bps")
            nc.vector.tensor_copy(bps[:], psB[:])
            # w = G1^T (G2^T bp) via two (lhsT-matmul, diag-mask, selector) rounds
            ws = bps
            for h_ in (72, 0):
                psS = ps_tp.tile([72, 8], F32, tag="tp", name="psS")
                nc.tensor.matmul(out=psS[:], lhsT=pboth[:, h_:h_ + 72],
                                 rhs=ws[:], start=True, stop=True)
                sm = work.tile([72, 8], BF16, tag="sm", name="sm")
                nc.vector.tensor_tensor(sm[:], psS[:], dgm[:], op=OP.mult)
                psY = ps_b.tile([T, 8], F32, tag="stb", name="psY")
                nc.tensor.matmul(out=psY[:], lhsT=selm[:], rhs=sm[:],
                                 start=True, stop=True)
                ws = work.tile([T, 8], BF16, tag="ws", name="ws")
                nc.vector.tensor_copy(ws[:], psY[:])
            ab = work.tile([T, 8], F32, tag="ab")
            nc.vector.tensor_tensor(ab[:], tmpAB[:, 0:8], ws[:], op=OP.mult)
            psZ = ps_tp.tile([1, 8], F32, tag="tp")
            nc.tensor.matmul(out=psZ[:], lhsT=ones9[:], rhs=ab[:],
                             start=True, stop=True)
            lz = pers.tile([1, 8], F32, tag="lz")
            nc.scalar.activation(lz[:], psZ[:], AF.Ln)

            # ---- final assembly: sum_b lz - gold_emis + 8*256*mu ----
            gtot = pers.tile([1, 1], F32, tag="gtot")
            nc.gpsimd.tensor_reduce(gtot[:], gjunk[:],
                                    axis=mybir.AxisListType.XYZWC, op=OP.add)
            red = pers.tile([1, 1], F32, tag="red")
            nc.vector.tensor_reduce(red[:], lz[:], axis=mybir.AxisListType.X,
                                    op=OP.add)
            diff = pers.tile([1, 1], F32, tag="diff")
            nc.vector.tensor_tensor(diff[:], red[:], gtot[:], op=OP.subtract)
            outc = pers.tile([1, 1], F32, tag="outc")
            nc.vector.tensor_scalar_add(outc[:], diff[:], float(BL * S * MU))
            nc.sync.dma_start(out_d[:], outc[:])

    nc.finalize()
    return nc


def _perm_scale_cols(w):
    """[*, 4HD] -> gate-chunk permuted cols, g-gate x2."""
    wc = w.reshape(w.shape[0], 8, 128)[:, PERM, :].copy()
    wc[:, 6:8, :] *= 2.0  # g-gate: tanh(g) = 2*sigmoid(2g) - 1
    return np.ascontiguousarray(wc.reshape(w.shape[0], 4 * HD))


def _prep_inputs(x, tags, crf_mask, embedding, W_ih_f, W_hh_f, b_f, W_ih_b,
                 W_hh_b, b_b, W_tag, b_tag, transitions, start_trans, end_trans):
    """Host-side sharding + layout prep. Pure reformatting / dtype casts."""
    x = np.asarray(x).astype(np.int32)
    tags = np.asarray(tags).astype(np.int64)
    mask = np.asarray(crf_mask)
    assert mask.all(), "kernel specialized to all-ones crf_mask"
    embedding = np.ascontiguousarray(np.asarray(embedding, dtype=np.float32))

    wih = {"f": _perm_scale_cols(np.asarray(W_ih_f, np.float32).T).astype(NPBF16),
           "b": _perm_scale_cols(np.asarray(W_ih_b, np.float32).T).astype(NPBF16)}
    whh = {"f": _perm_scale_cols(np.asarray(W_hh_f, np.float32).T).astype(NPBF16),
           "b": _perm_scale_cols(np.asarray(W_hh_b, np.float32).T).astype(NPBF16)}
    brs = {}
    for d, b_ in (("f", b_f), ("b", b_b)):
        bv = np.asarray(b_, np.float32).reshape(8, 128)[PERM, :].copy()
        bv[6:8, :] *= 2.0
        brs[d] = np.ascontiguousarray(bv.T)  # [128, 8]
    # W_tag [9, 512] -> [128 p, 2 d, 2 k, 9] bf16
    wt = np.asarray(W_tag, np.float32).T                      # [512, 9]
    wt = wt.reshape(2, 2, 128, T).transpose(2, 0, 1, 3)       # [128, d, k, 9]
    wtag8 = np.ascontiguousarray(wt.reshape(128, 2 * 2 * T)).astype(NPBF16)
    btag = np.asarray(b_tag, np.float32).reshape(T, 1)
    btagmu = (btag - MU).astype(np.float32)
    trans = np.asarray(transitions, np.float32)
    expT = np.ascontiguousarray(np.exp(trans)).astype(NPBF16)
    expTT = np.ascontiguousarray(np.exp(trans).T).astype(NPBF16)
    expsv = np.exp(np.asarray(start_trans, np.float32)).reshape(T, 1)
    expev = np.exp(np.asarray(end_trans, np.float32)).reshape(T, 1)
    idf32 = np.eye(128, dtype=np.float32)
    idf16 = np.eye(128, dtype=np.float16)
    jj = np.arange(72) // 8
    bb72 = np.arange(72) % 8
    selm = (jj[:, None] == np.arange(T)[None, :]).astype(NPBF16)      # [72, 9]
    dgm = (bb72[:, None] == np.arange(8)[None, :]).astype(NPBF16)     # [72, 8]

    shared = {
        "emb": embedding, "wih_f": wih["f"], "wih_b": wih["b"],
        "whh_f": whh["f"], "whh_b": whh["b"], "br_f": brs["f"],
        "br_b": brs["b"], "wtag8": wtag8, "btag": btag, "btagmu": btagmu,
        "expT": expT, "expTT": expTT, "exps": expsv, "expe": expev,
        "selm": selm, "dgm": dgm, "idf32": idf32, "idf16": idf16,
    }

    # host-side gold transition/start/end score per core
    start = np.asarray(start_trans, np.float64)
    end = np.asarray(end_trans, np.float64)
    transd = np.asarray(transitions, np.float64)
    in_maps, gold_tr = [], []
    tt = np.arange(TOK) // BL   # token -> t
    bb = np.arange(TOK) % BL    # token -> local b
    for c in range(NCORES):
        xc = x[c * BL:(c + 1) * BL]          # [8, 256]
        tc_ = tags[c * BL:(c + 1) * BL]      # [8, 256]
        idx = xc[bb, tt].astype(np.int32)    # [2048] token-major (t,b)
        idx_h = np.ascontiguousarray(idx.reshape(NCH, 128).T)  # [128, NCH]
        tag_tok = tc_[bb, tt]                # [2048]
        ohc = (tag_tok[None, :] == np.arange(T)[:, None]).astype(np.float32)
        m = dict(shared)
        m["idx"] = idx_h
        m["ohc"] = np.ascontiguousarray(ohc)
        in_maps.append(m)
        gold_tr.append(
            float(start[tc_[:, 0]].sum() + end[tc_[:, -1]].sum()
                  + transd[tc_[:, :-1], tc_[:, 1:]].sum())
        )
    return in_maps, gold_tr


def _run(inputs, trace=False):
    nc = _build(S)
    in_maps, gold_tr = _prep_inputs(**inputs)
    res = run_bass_kernel_spmd(
        nc, in_maps, core_ids=list(range(NCORES)), trace=trace
    )
    total = np.float64(0.0)
    for c in range(NCORES):
        total += np.float64(res.results[c]["out"][0, 0]) - gold_tr[c]
    return np.float32(total), res


def kernel(**inputs) -> np.ndarray:
    out, _ = _run(inputs, trace=False)
    return out


# revision 23
# speedup vs baseline: 1.0453x; 1.0453x over previous
"""BiLSTM-CRF NLL kernel for 8 Trainium2 NeuronCores.

Contract: kernel(**inputs) takes the FULL unsharded inputs (as produced by the
reference setup_inputs()) and returns the FULL output (a float32 scalar).

Sharding (hardcoded): data-parallel over batch. B=64 -> 8 shards of 8
sequences; params replicated. Each core computes sum_b(logZ_b) - sum emis[gold]
for its 8 sequences; the host adds the (host-computable) gold
transition/start/end score and sums the 8 partial scalars.

Per-core pipeline:
  0. embedding gather via indirect DMA ([128, E] token chunks), PE-transposed
     to xT [E, tokens] in bf16.
  1. input projections g_ih = W_ih @ x + b, bf16 matmuls with N=256,
     stored f16, gate chunks permuted to (i,i,f,f,o,o,g,g), g rows x2 extra.
     Emission is PACED: ~2 PE ops per recurrence step so phase-1 never stalls
     the recurrence chain.
  2. the two LSTM recurrences interleaved. Per step and direction: one f16
     identity-matmul preloads g_ih into PSUM (one step ahead); 16 bf16
     [128,128]x[128,8] matmuls accumulate W_hh @ h (these pipeline at ~27ns
     pitch on the PE); sigmoid on ACT covers all four gates (tanh(g) via
     2*sig(2g)-1); cell update on DVE; tanh on ACT; h = o*tanh(c) written
     bf16 by GPSIMD straight into the k-major h history buffer that serves
     as both the next-step matmul operand and the emission operand.
  3. emissions transposed [9, tokens]: per 512-token group four bf16 matmuls
     (dir x k-plane, contiguous rhs thanks to the k-major h layout);
     E = exp(psum + b_tag - mu) in one ACT; raw emissions = psum + b_tag
     on DVE (for the gold score).
  4. gold emission score: (emisraw * onehot) with accum_out on GPSIMD,
     overlapped with the CRF chains.
  5. CRF in exp space, bf16, as FOUR concurrent chains (serial depth 84):
     alpha (t=0..83) and beta (t=255..172) vector chains plus two per-batch
     9x9 transfer-matrix product chains covering the middle; the combine
     contracts alpha^T G1^T G2^T (M beta) with two lhsT-matmuls using a
     diagonal-batch mask + selector matmul. logZ = log(sum) + 256*mu.
     The gold emission score runs on GPSIMD concurrently with the CRF.
"""

import functools
import math
import os
import sys

import numpy as np

for _p in ("/opt/trn_rl_repo", "/opt/pypackages"):
    if _p not in sys.path and os.path.isdir(_p):
        sys.path.append(_p)

import ml_dtypes  # noqa: E402

import concourse.bass as bass  # noqa: E402
import concourse.mybir as mybir  # noqa: E402
import concourse.tile as tile  # noqa: E402
from concourse import bacc  # noqa: E402
from concourse.bass import IndirectOffsetOnAxis  # noqa: E402
from concourse.bass_utils import run_bass_kernel_spmd  # noqa: E402

F32 = mybir.dt.float32
F16 = mybir.dt.float16
BF16 = mybir.dt.bfloat16
FP8 = mybir.dt.float8e4
I32 = mybir.dt.int32
AF = mybir.ActivationFunctionType
OP = mybir.AluOpType
DR = mybir.MatmulPerfMode.DoubleRowSwInterleave
NPFP8 = mybir.dt.np(FP8)
NPBF16 = ml_dtypes.bfloat16

# Problem constants (hardcoded per the task contract).
B, S, V, E, H, T = 64, 256, 50000, 256, 512, 9
HD = H // 2               # 256 per-direction hidden
NCORES = 8
BL = B // NCORES          # 8 sequences per core
TOK = BL * S              # 2048 tokens per core
NCH = TOK // 128          # 16 gather chunks of 128 tokens
MU = math.log(9.0)        # exp-space drift compensation, cancels exactly
# gate chunk permutation: original (i0 i1 f0 f1 g0 g1 o0 o1) -> (i i f f o o g g)
PERM = [0, 1, 2, 3, 6, 7, 4, 5]
SLOTS = S + 1             # h history slots per direction


def _hslot(hall_v, s):
    """[128, 2, 8] view of history slot s (k-major hall)."""
    return hall_v[:, :, s, :]


def _emit_preload(nc, d, t, gih, idf16, ps_pool):
    """Start step-t PSUM with g_ih(+bias) via identity matmul (h-independent)."""
    ps = ps_pool[d].tile([128, 64], F32, tag=f"st{d}", name=f"ps{d}")
    nc.tensor.matmul(
        out=ps[:, :],
        lhsT=idf16[:],
        rhs=gih[d][:, t * 64:(t + 1) * 64],
        start=True,
        stop=False,
        skip_group_check=True,
    )
    return ps


def _emit_wmms(nc, d, t, ps, whhv, hall_v):
    rd = t if d == "f" else t + 1
    rhs = _hslot(hall_v[d], rd)
    for m in range(8):
        for k in range(2):
            nc.tensor.matmul(
                out=ps[:, m * 8:(m + 1) * 8],
                lhsT=whhv[d][k][:, m * 128:(m + 1) * 128],
                rhs=rhs[:, k],
                start=False,
                stop=(m == 7 and k == 1),
                skip_group_check=True,
            )


def _emit_tail(nc, d, t, ps, hall_v, c_state, work):
    wr = t + 1 if d == "f" else t
    # g-gate preacts are pre-scaled x2 on host: one sigmoid covers all four
    # gates: sig(i), sig(f), sig(o), sig(2g); tanh(g) = 2*sig(2g) - 1.
    sig = work.tile([128, 64], BF16, tag=f"sig{d}", name=f"sig{d}")
    nc.scalar.activation(sig[:], ps[:, :], AF.Sigmoid)
    u = work.tile([128, 16], BF16, tag=f"u{d}", name=f"u{d}")
    nc.vector.scalar_tensor_tensor(
        u[:], sig[:, 48:64], 0.5, sig[:, 0:16], op0=OP.subtract, op1=OP.mult
    )
    v = work.tile([128, 16], BF16, tag=f"v{d}", name=f"v{d}")
    nc.vector.tensor_tensor(v[:], sig[:, 16:32], c_state[d][:], op=OP.mult)
    nc.vector.scalar_tensor_tensor(
        c_state[d][:], u[:], 2.0, v[:], op0=OP.mult, op1=OP.add
    )
    tcn = work.tile([128, 16], BF16, tag=f"tc{d}", name=f"tc{d}")
    nc.scalar.activation(tcn[:], c_state[d][:], AF.Tanh)
    ogate = sig[:].rearrange("p (m k b) -> p m k b", m=4, k=2, b=8)[:, 2]
    tcv = tcn[:].rearrange("p (k b) -> p k b", k=2, b=8)
    nc.vector.tensor_tensor(_hslot(hall_v[d], wr), ogate, tcv, op=OP.mult)
    return tcn


@functools.lru_cache(maxsize=2)
def _build(seq_len=S):
    """Build the Bass program (same SPMD program for all 8 cores)."""
    assert seq_len == S, "builder is specialized to S=256"

    nc = bacc.Bacc("TRN2", target_bir_lowering=False, debug=False)

    # ---- DRAM I/O ----
    emb_d = nc.dram_tensor("emb", [V, E], F32, kind="ExternalInput")
    idx_d = nc.dram_tensor("idx", [128, NCH], I32, kind="ExternalInput")
    wih_d = {d: nc.dram_tensor(f"wih_{d}", [E, 4 * HD], BF16, kind="ExternalInput")
             for d in "fb"}
    whh_d = {d: nc.dram_tensor(f"whh_{d}", [HD, 4 * HD], BF16, kind="ExternalInput")
             for d in "fb"}
    br_d = {d: nc.dram_tensor(f"br_{d}", [128, 8], F32, kind="ExternalInput")
            for d in "fb"}
    wtag_d = nc.dram_tensor("wtag8", [128, 2 * 2 * T], BF16, kind="ExternalInput")
    btag_d = nc.dram_tensor("btag", [T, 1], F32, kind="ExternalInput")
    btagmu_d = nc.dram_tensor("btagmu", [T, 1], F32, kind="ExternalInput")
    expT_d = nc.dram_tensor("expT", [T, T], BF16, kind="ExternalInput")
    expTT_d = nc.dram_tensor("expTT", [T, T], BF16, kind="ExternalInput")
    exps_d = nc.dram_tensor("exps", [T, 1], F32, kind="ExternalInput")
    expe_d = nc.dram_tensor("expe", [T, 1], F32, kind="ExternalInput")
    ohc_d = nc.dram_tensor("ohc", [T, TOK], F32, kind="ExternalInput")
    selm_d = nc.dram_tensor("selm", [72, T], BF16, kind="ExternalInput")
    dgm_d = nc.dram_tensor("dgm", [72, 8], BF16, kind="ExternalInput")
    idf32_d = nc.dram_tensor("idf32", [128, 128], F32, kind="ExternalInput")
    idf16_d = nc.dram_tensor("idf16", [128, 128], F16, kind="ExternalInput")
    out_d = nc.dram_tensor("out", [1, 1], F32, kind="ExternalOutput")

    with tile.TileContext(nc) as tc:
        with (
            tc.tile_pool(name="pers", bufs=1) as pers,
            tc.tile_pool(name="work", bufs=3) as work,
            tc.tile_pool(name="psbig", bufs=2, space="PSUM") as ps_big,
            tc.tile_pool(name="pstp", bufs=2, space="PSUM") as ps_tp,
            tc.tile_pool(name="psf", bufs=2, space="PSUM") as ps_f,
            tc.tile_pool(name="psb", bufs=2, space="PSUM") as ps_b,
        ):
            ps_pool = {"f": ps_f, "b": ps_b}

            # ---- persistent SBUF ----
            idx_sb = pers.tile([128, NCH], I32, tag="idx")
            nc.sync.dma_start(idx_sb[:], idx_d[:])
            # gathers first: the GPSIMD queue starts with the indirect DMAs
            xg = pers.tile([128, NCH * E], F32, tag="xg")
            xT = [pers.tile([128, TOK], BF16, tag=f"xT{k}", name=f"xT{k}")
                  for k in range(2)]
            gorder = []
            for j in range(NCH // 4):
                gorder += [2 * j, 2 * j + 1, NCH - 2 - 2 * j, NCH - 1 - 2 * j]
            for ch in gorder:
                nc.gpsimd.indirect_dma_start(
                    out=xg[:, ch * E:(ch + 1) * E],
                    out_offset=None,
                    in_=emb_d[:],
                    in_offset=IndirectOffsetOnAxis(ap=idx_sb[:, ch:ch + 1], axis=0),
                )
            idf32 = pers.tile([128, 128], F32, tag="idf32")
            nc.sync.dma_start(idf32[:], idf32_d[:])
            idf16 = pers.tile([128, 128], F16, tag="idf16")
            nc.sync.dma_start(idf16[:], idf16_d[:])

            wih, whh, whhv, br, gih, hall, hall_v, c_state = ({} for _ in range(8))
            for d in "fb":
                wih[d] = [pers.tile([128, 4 * HD], BF16, tag=f"wih{d}{k}",
                                    name=f"wih{d}{k}") for k in range(2)]
                for k in range(2):
                    nc.sync.dma_start(wih[d][k][:], wih_d[d][k * 128:(k + 1) * 128, :])
                whh[d] = [pers.tile([128, 4 * HD], BF16, tag=f"whh{d}{k}",
                                    name=f"whh{d}{k}") for k in range(2)]
                for k in range(2):
                    nc.sync.dma_start(whh[d][k][:], whh_d[d][k * 128:(k + 1) * 128, :])
                whhv[d] = whh[d]
                br[d] = pers.tile([128, 8], F32, tag=f"br{d}", name=f"br{d}")
                nc.sync.dma_start(br[d][:], br_d[d][:])
                gih[d] = pers.tile([128, S * 64], F16, tag=f"gih{d}", name=f"gih{d}")
                hall[d] = pers.tile([128, 2 * SLOTS * 8], BF16, tag=f"hall{d}",
                                    name=f"hall{d}")
                hall_v[d] = hall[d][:].rearrange("p (k s b) -> p k s b",
                                                 k=2, s=SLOTS, b=8)
                c_state[d] = pers.tile([128, 16], F32, tag=f"c{d}", name=f"c{d}")
                nc.vector.memset(c_state[d][:], 0.0)
            # zero initial h slots (fwd reads slot 0, bwd reads slot S)
            nc.vector.memset(_hslot(hall_v["f"], 0), 0.0)
            nc.vector.memset(_hslot(hall_v["b"], S), 0.0)

            wtag8 = pers.tile([128, 2 * 2 * T], BF16, tag="wtag8")
            nc.sync.dma_start(wtag8[:], wtag_d[:])
            wtagv = wtag8[:].rearrange("p (d k t) -> p d k t", d=2, k=2, t=T)
            btag = pers.tile([T, 1], F32, tag="btag")
            nc.sync.dma_start(btag[:], btag_d[:])
            btagmu = pers.tile([T, 1], F32, tag="btagmu")
            nc.sync.dma_start(btagmu[:], btagmu_d[:])
            expTm = pers.tile([T, T], BF16, tag="expTm")
            nc.sync.dma_start(expTm[:], expT_d[:])
            expTTm = pers.tile([T, T], BF16, tag="expTTm")
            nc.sync.dma_start(expTTm[:], expTT_d[:])
            exps = pers.tile([T, 1], F32, tag="exps")
            nc.sync.dma_start(exps[:], exps_d[:])
            expe = pers.tile([T, 1], F32, tag="expe")
            nc.sync.dma_start(expe[:], expe_d[:])
            ohc = pers.tile([T, TOK], F32, tag="ohc")
            nc.sync.dma_start(ohc[:], ohc_d[:])
            ones9 = pers.tile([T, 1], F32, tag="ones9")
            nc.vector.memset(ones9[:], 1.0)
            # warm the Exp/Ln ACT tables so the scheduler's mid-loop hoist of
            # the emission exp never pays a table load on the critical path
            warm = pers.tile([1, 1], F32, tag="warm")
            nc.scalar.activation(warm[:], ones9[0:1, 0:1], AF.Exp)
            nc.scalar.activation(warm[:], ones9[0:1, 0:1], AF.Ln)
            selm = pers.tile([72, T], BF16, tag="selm")
            nc.sync.dma_start(selm[:], selm_d[:])
            dgm = pers.tile([72, 8], BF16, tag="dgm")
            nc.sync.dma_start(dgm[:], dgm_d[:])

            # ---- phase 1 as a paced op queue ----
            # chunk = 256 tokens = 32 t-slots; ops are closures, ~1 PE op each.
            transposed = set()

            def ops_transpose(gch):
                if gch in transposed:
                    return
                transposed.add(gch)
                for k in range(2):
                    def op(gch=gch, k=k):
                        pst = ps_tp.tile([128, 128], F32, tag="tp", name="tp")
                        nc.tensor.transpose(
                            out=pst[:],
                            in_=xg[:, gch * E + k * 128: gch * E + (k + 1) * 128],
                            identity=idf32[:],
                        )
                        nc.vector.tensor_copy(xT[k][:, gch * 128:(gch + 1) * 128],
                                              pst[:])
                    yield op

            def ops_chunk(d, c, half=None):
                # 256-token chunk c (t-slots 32c..32c+31), direction d.
                # half=0/1 emits only the first/second 128 tokens (16 slots).
                gchs = [2 * c, 2 * c + 1] if half is None else [2 * c + half]
                for g in gchs:
                    yield from ops_transpose(g)
                ntok = 256 if half is None else 128
                t00 = c * 32 + (0 if not half else 16)
                c00 = c * 256 + (0 if not half else 128)
                for m in range(8):
                    def op(d=d, m=m, ntok=ntok, t00=t00, c00=c00):
                        psg = ps_big.tile([128, ntok], F32, tag="big", name="psg")
                        for k in range(2):
                            nc.tensor.matmul(
                                out=psg[:],
                                lhsT=wih[d][k][:, m * 128:(m + 1) * 128],
                                rhs=xT[k][:, c00:c00 + ntok],
                                start=(k == 0),
                                stop=(k == 1),
                            )
                        dst = gih[d][:].rearrange(
                            "p (t m b) -> p t m b", t=S, m=8, b=8
                        )[:, t00:t00 + ntok // 8, m, :]
                        srcv = psg[:].rearrange("p (t b) -> p t b",
                                                t=ntok // 8, b=8)
                        if m % 2 == 0:
                            nc.vector.tensor_scalar_add(dst, srcv, br[d][:, m:m + 1])
                        else:
                            nc.scalar.activation(dst, srcv, AF.Identity,
                                                 bias=br[d][:, m:m + 1])
                    yield op

            NCHK = S // 32  # 8 chunks per direction
            # prelude: first fwd chunk + last bwd chunk fully
            # (list before building the queue: listing marks transposes done)
            prelude = (list(ops_chunk("f", 0)) + list(ops_chunk("b", NCHK - 1))
                       + list(ops_chunk("f", 1)) + list(ops_chunk("b", NCHK - 2)))
            p1_queue = []
            for j in range(2, NCHK):
                p1_queue += list(ops_chunk("f", j))
                p1_queue += list(ops_chunk("b", NCHK - 1 - j))
            p1_pos = 0
            for op in prelude:
                op()

            # ---- phase 2: the two LSTM recurrences, phase-1 paced in ----
            ps_cur = {"f": _emit_preload(nc, "f", 0, gih, idf16, ps_pool),
                      "b": _emit_preload(nc, "b", S - 1, gih, idf16, ps_pool)}
            for t in range(S):
                for d in "fb":
                    tok = t if d == "f" else S - 1 - t
                    _emit_wmms(nc, d, tok, ps_cur[d], whhv, hall_v)
                for _ in range(2):
                    if p1_pos < len(p1_queue):
                        p1_queue[p1_pos]()
                        p1_pos += 1
                ps_nxt = {}
                if t + 1 < S:
                    ps_nxt = {
                        "f": _emit_preload(nc, "f", t + 1, gih, idf16, ps_pool),
                        "b": _emit_preload(nc, "b", S - 2 - t, gih, idf16,
                                           ps_pool),
                    }
                for d in "fb":
                    tok = t if d == "f" else S - 1 - t
                    _emit_tail(nc, d, tok, ps_cur[d], hall_v, c_state, work)
                ps_cur = ps_nxt

            # ---- phase 3: emissions via fp8 DoubleRow + fused exp ----
            emisraw = pers.tile([T, TOK], F32, tag="emisraw")
            ebuf = pers.tile([T, TOK], F32, tag="ebuf")
            hflat = {d: hall[d][:].rearrange("p (k sb) -> p k sb", k=2)
                     for d in "fb"}
            for n in range(4):
                pse = ps_big.tile([T, 512], F32, tag="big", name="pse")
                # fwd h_t lives at slot t+1; bwd h_t at slot t
                lo = {"f": (n * 64 + 1) * 8, "b": (n * 64) * 8}
                for di, d in enumerate("fb"):
                    for k in range(2):
                        nc.tensor.matmul(
                            out=pse[:], lhsT=wtagv[:, di, k],
                            rhs=hflat[d][:, k, lo[d]:lo[d] + 512],
                            start=(di == 0 and k == 0),
                            stop=(di == 1 and k == 1),
                        )
                nc.scalar.activation(ebuf[:, n * 512:(n + 1) * 512], pse[:],
                                     AF.Exp, bias=btagmu[:, 0:1])
                nc.vector.tensor_scalar_add(
                    emisraw[:, n * 512:(n + 1) * 512], pse[:], btag[:, 0:1]
                )

            # ---- phase 4: gold emission score on GPSIMD (overlaps CRF) ----
            gjunk = pers.tile([T, TOK], F32, tag="gjunk")
            for n in range(4):
                nc.gpsimd.tensor_tensor(
                    gjunk[:, n * 512:(n + 1) * 512],
                    emisraw[:, n * 512:(n + 1) * 512],
                    ohc[:, n * 512:(n + 1) * 512], op=OP.mult,
                )

            # ---- phase 5: CRF, four concurrent chains (depth 64) ----
            # A: alpha over t=0..63.  B: beta over t=255..192.
            # M1/M2: per-batch 9x9 transfer products over t=64..127 / 128..191,
            # stored [9, (j0, b)] j-major.  alpha_191 = P2 P1 a63;
            # logZ = sum_i alpha_191[i] * (M b_192)[i].
            e3 = ebuf[:].rearrange("p (t b) -> p t b", t=S, b=8)
            NQA = 84   # A/B chain depth (cheap rounds)
            NQM1 = 44  # M1 depth
            NQM2 = S - 2 * NQA - NQM1  # 44: M2 depth
            tmpAB = work.tile([T, 16], BF16, tag="tmpAB")
            nc.vector.tensor_scalar(
                tmpAB[:, 0:8], ebuf[:, 0:8], scalar1=exps[:, 0:1], scalar2=None,
                op0=OP.mult,
            )
            nc.vector.tensor_scalar(
                tmpAB[:, 8:16], ebuf[:, (S - 1) * 8:S * 8],
                scalar1=expe[:, 0:1], scalar2=None, op0=OP.mult,
            )
            # seeds: P = M^T * E_t (broadcast over j0 / b), t = 64 and 128
            pboth = work.tile([T, 144], BF16, tag="pboth")
            mtb = expTTm[:].unsqueeze(2).broadcast_to([T, T, 8])
            for h_, t0 in ((0, NQA), (72, NQA + NQM1)):
                eb = e3[:, t0:t0 + 1, :].broadcast_to([T, T, 8])
                nc.vector.tensor_tensor(
                    pboth[:, h_:h_ + 72].rearrange("p (j b) -> p j b", j=T, b=8),
                    mtb, eb, op=OP.mult,
                )
            for i in range(1, NQA):  # A: t=i; B: t=255-i; M: shorter chains
                tA = i
                tB = S - 1 - i
                psAB = ps_f.tile([T, 16], F32, tag="stf", name="psAB")
                nc.tensor.matmul(out=psAB[:, 0:8], lhsT=expTm[:],
                                 rhs=tmpAB[:, 0:8], start=True, stop=True)
                nc.tensor.matmul(out=psAB[:, 8:16], lhsT=expTTm[:],
                                 rhs=tmpAB[:, 8:16], start=True, stop=True)
                do1, do2 = i < NQM1, i < NQM2
                if do1 or do2:
                    psM = ps_big.tile([T, 144], F32, tag="big", name="psM")
                    if do1:
                        nc.tensor.matmul(out=psM[:, 0:72], lhsT=expTm[:],
                                         rhs=pboth[:, 0:72], start=True, stop=True)
                    if do2:
                        nc.tensor.matmul(out=psM[:, 72:144], lhsT=expTm[:],
                                         rhs=pboth[:, 72:144], start=True, stop=True)
                tmpAB = work.tile([T, 16], BF16, tag="tmpAB")
                nc.vector.tensor_tensor(
                    tmpAB[:], psAB[:], e3[:, tA:tB + 1:(tB - tA), :],
                    op=OP.mult,
                )
                if do1 or do2:
                    pprev, pboth = pboth, work.tile([T, 144], BF16, tag="pboth")
                    for q, do in ((0, do1), (1, do2)):
                        if not do:
                            nc.vector.tensor_copy(
                                pboth[:, q * 72:(q + 1) * 72],
                                pprev[:, q * 72:(q + 1) * 72])
                            continue
                        t_ = NQA + q * NQM1 + i
                        eM = e3[:, t_:t_ + 1, :]
                        eMb = eM.broadcast_to([T, T, 8])
                        nc.vector.tensor_tensor(
                            pboth[:, q * 72:(q + 1) * 72].rearrange(
                                "p (j b) -> p j b", j=T, b=8),
                            psM[:, q * 72:(q + 1) * 72].rearrange(
                                "p (j b) -> p j b", j=T, b=8),
                            eMb, op=OP.mult,
                        )
            # bp = M @ b_192
            psB = ps_b.tile([T, 8], F32, tag="stb")
            nc.tensor.matmul(out=psB[:], lhsT=expTTm[:], rhs=tmpAB[:, 8:16],
                             start=True, stop=True)
            bps = work.tile([T, 8], BF16, tag="bps")
            nc.vector.tensor_copy(bps[:], psB[:])
            # w = G1^T (G2^T bp) via two (lhsT-matmul, diag-mask, selector) rounds
            ws = bps
            for h_ in (72, 0):
                psS = ps_tp.tile([72, 8], F32, tag="tp", name="psS")
                nc.tensor.matmul(out=psS[:], lhsT=pboth[:, h_:h_ + 72],
                                 rhs=ws[:], start=True, stop=True)
                sm = work.tile([72, 8], BF16, tag="sm", name="sm")
                nc.vector.tensor_tensor(sm[:], psS[:], dgm[:], op=OP.mult)
                psY = ps_b.tile([T, 8], F32, tag="stb", name="psY")
                nc.tensor.matmul(out=psY[:], lhsT=selm[:], rhs=sm[:],
                                 start=True, stop=True)
                ws = work.tile([T, 8], BF16, tag="ws", name="ws")
                nc.vector.tensor_copy(ws[:], psY[:])
            ab = work.tile([T, 8], F32, tag="ab")
            nc.vector.tensor_tensor(ab[:], tmpAB[:, 0:8], ws[:], op=OP.mult)
            psZ = ps_tp.tile([1, 8], F32, tag="tp")
            nc.tensor.matmul(out=psZ[:], lhsT=ones9[:], rhs=ab[:],
                             start=True, stop=True)
            lz = pers.tile([1, 8], F32, tag="lz")
            nc.scalar.activation(lz[:], psZ[:], AF.Ln)

            # ---- final assembly: sum_b lz - gold_emis + 8*256*mu ----
            gtot = pers.tile([1, 1], F32, tag="gtot")
            nc.gpsimd.tensor_reduce(gtot[:], gjunk[:],
                                    axis=mybir.AxisListType.XYZWC, op=OP.add)
            red = pers.tile([1, 1], F32, tag="red")
            nc.vector.tensor_reduce(red[:], lz[:], axis=mybir.AxisListType.X,
                                    op=OP.add)
            diff = pers.tile([1, 1], F32, tag="diff")
            nc.vector.tensor_tensor(diff[:], red[:], gtot[:], op=OP.subtract)
            outc = pers.tile([1, 1], F32, tag="outc")
            nc.vector.tensor_scalar_add(outc[:], diff[:], float(BL * S * MU))
            nc.sync.dma_start(out_d[:], outc[:])

    nc.finalize()
    return nc


def _perm_scale_cols(w):
    """[*, 4HD] -> gate-chunk permuted cols, g-gate x2."""
    wc = w.reshape(w.shape[0], 8, 128)[:, PERM, :].copy()
    wc[:, 6:8, :] *= 2.0  # g-gate: tanh(g) = 2*sigmoid(2g) - 1
    return np.ascontiguousarray(wc.reshape(w.shape[0], 4 * HD))


def _prep_inputs(x, tags, crf_mask, embedding, W_ih_f, W_hh_f, b_f, W_ih_b,
                 W_hh_b, b_b, W_tag, b_tag, transitions, start_trans, end_trans):
    """Host-side sharding + layout prep. Pure reformatting / dtype casts."""
    x = np.asarray(x).astype(np.int32)
    tags = np.asarray(tags).astype(np.int64)
    mask = np.asarray(crf_mask)
    assert mask.all(), "kernel specialized to all-ones crf_mask"
    embedding = np.ascontiguousarray(np.asarray(embedding, dtype=np.float32))

    wih = {"f": _perm_scale_cols(np.asarray(W_ih_f, np.float32).T).astype(NPBF16),
           "b": _perm_scale_cols(np.asarray(W_ih_b, np.float32).T).astype(NPBF16)}
    whh = {"f": _perm_scale_cols(np.asarray(W_hh_f, np.float32).T).astype(NPBF16),
           "b": _perm_scale_cols(np.asarray(W_hh_b, np.float32).T).astype(NPBF16)}
    brs = {}
    for d, b_ in (("f", b_f), ("b", b_b)):
        bv = np.asarray(b_, np.float32).reshape(8, 128)[PERM, :].copy()
        bv[6:8, :] *= 2.0
        brs[d] = np.ascontiguousarray(bv.T)  # [128, 8]
    # W_tag [9, 512] -> [128 p, 2 d, 2 k, 9] bf16
    wt = np.asarray(W_tag, np.float32).T                      # [512, 9]
    wt = wt.reshape(2, 2, 128, T).transpose(2, 0, 1, 3)       # [128, d, k, 9]
    wtag8 = np.ascontiguousarray(wt.reshape(128, 2 * 2 * T)).astype(NPBF16)
    btag = np.asarray(b_tag, np.float32).reshape(T, 1)
    btagmu = (btag - MU).astype(np.float32)
    trans = np.asarray(transitions, np.float32)
    expT = np.ascontiguousarray(np.exp(trans)).astype(NPBF16)
    expTT = np.ascontiguousarray(np.exp(trans).T).astype(NPBF16)
    expsv = np.exp(np.asarray(start_trans, np.float32)).reshape(T, 1)
    expev = np.exp(np.asarray(end_trans, np.float32)).reshape(T, 1)
    idf32 = np.eye(128, dtype=np.float32)
    idf16 = np.eye(128, dtype=np.float16)
    jj = np.arange(72) // 8
    bb72 = np.arange(72) % 8
    selm = (jj[:, None] == np.arange(T)[None, :]).astype(NPBF16)      # [72, 9]
    dgm = (bb72[:, None] == np.arange(8)[None, :]).astype(NPBF16)     # [72, 8]

    shared = {
        "emb": embedding, "wih_f": wih["f"], "wih_b": wih["b"],
        "whh_f": whh["f"], "whh_b": whh["b"], "br_f": brs["f"],
        "br_b": brs["b"], "wtag8": wtag8, "btag": btag, "btagmu": btagmu,
        "expT": expT, "expTT": expTT, "exps": expsv, "expe": expev,
        "selm": selm, "dgm": dgm, "idf32": idf32, "idf16": idf16,
    }

    # host-side gold transition/start/end score per core
    start = np.asarray(start_trans, np.float64)
    end = np.asarray(end_trans, np.float64)
    transd = np.asarray(transitions, np.float64)
    in_maps, gold_tr = [], []
    tt = np.arange(TOK) // BL   # token -> t
    bb = np.arange(TOK) % BL    # token -> local b
    for c in range(NCORES):
        xc = x[c * BL:(c + 1) * BL]          # [8, 256]
        tc_ = tags[c * BL:(c + 1) * BL]      # [8, 256]
        idx = xc[bb, tt].astype(np.int32)    # [2048] token-major (t,b)
        idx_h = np.ascontiguousarray(idx.reshape(NCH, 128).T)  # [128, NCH]
        tag_tok = tc_[bb, tt]                # [2048]
        ohc = (tag_tok[None, :] == np.arange(T)[:, None]).astype(np.float32)
        m = dict(shared)
        m["idx"] = idx_h
        m["ohc"] = np.ascontiguousarray(ohc)
        in_maps.append(m)
        gold_tr.append(
            float(start[tc_[:, 0]].sum() + end[tc_[:, -1]].sum()
                  + transd[tc_[:, :-1], tc_[:, 1:]].sum())
        )
    return in_maps, gold_tr


def _run(inputs, trace=False):
    nc = _build(S)
    in_maps, gold_tr = _prep_inputs(**inputs)
    res = run_bass_kernel_spmd(
        nc, in_maps, core_ids=list(range(NCORES)), trace=trace
    )
    total = np.float64(0.0)
    for c in range(NCORES):
        total += np.float64(res.results[c]["out"][0, 0]) - gold_tr[c]
    return np.float32(total), res


def kernel(**inputs) -> np.ndarray:
    out, _ = _run(inputs, trace=False)
    return out
